# revision 64
# baseline (speedup 1.0000x reference)
"""BarrierNet kernel for Trainium2 (Bass/Tile), 8-core data parallel.

Computes, for each batch row b:
  out[b] = [x31 (2) | G.flat (514) | h (257)]   (773 columns)

Key structure: for a row-tile of 128 rows, every output column is a dot
product of <= 15 per-row features with a per-column constant vector
(derived from the obstacle table), so the whole [128 x 773] tile is
produced by one PE matmul against a host-prebuilt RHS[15, 773] matrix.
Per-row features are computed elementwise with rows on partitions and
moved into lhsT layout with a per-tile PE transpose.

Precision/bandwidth: the matmul path (xT, weights, features, RHS) and
the DRAM output run in fp16 (l2 rel err ~4e-4 vs the 2e-2 gate); fp32
matmuls cost 4 PE cycles/row vs 1 for fp16, and fp16 stores halve the
~25 MB/core of output traffic (DMA floor ~35us/core). The host upcasts
the gathered fp16 output to f32.

Engine budget per core: only DVE and ACT can read PSUM on trn2, so the
~74k columns/core of PSUM drains (output copies, relus, lhsT copies)
are split across them: DVE gets PSUM bank A (SPLIT_COL cols/tile) +
relu2 + the fp16->fp16 lhsT copies (2x mode); ACT gets bank B
(pair-batched to amortize its ~240ns/instr fixed cost) + relu1 + the
trig/tanh ops. sin/cos use the native ACT Sin table (domain |x|<~4.2,
after DVE range reduction; cos via the half-angle identity), and
4*sigmoid(z) is computed as 2+2*tanh(z/2) so that one activation
function set (silu_and_others) serves the whole kernel. b1 is folded
into the W1 matmul via a ones row in xT; the x31/sigma biases enter
through a rank-1 accumulate matmul; W21|W22 run as one stacked matmul
whose [64, CH] outputs are partition-packed in pairs so relu2 drains
[128, CH] per instruction. GPSIMD (no PSUM access) carries the
SBUF-only opponent-feature math and the xj input DMA.

Host-side work is layout-only (transposes/replication of inputs plus
the tiny 256-row obstacle feature table).
"""

import os

import numpy as np

import concourse.bass as bass
import concourse.bacc as bacc
import concourse.mybir as mybir
import concourse.tile as tile
from concourse.bass_utils import run_bass_kernel_spmd

dt = mybir.dt
AF = mybir.ActivationFunctionType
ALU = mybir.AluOpType

B_TOT = 65536
N_IN = 8
NXT = N_IN + 1                 # xT rows: 8 inputs + ones row (b1 folding)
N_CORES = 8
BS = B_TOT // N_CORES          # 8192 rows per core
P = 128
NT = BS // P                   # 64 row tiles per core
N_OBS = 256
N_COLS = 2 + 2 * (N_OBS + 1) + (N_OBS + 1)   # 773
H1 = 128
H2 = 32
K_F = 15                       # used feature rows in the big matmul
F_SLOTS = 16                   # feature slot stride in Fbuf

NSPLIT = int(os.environ.get("BNET_NSPLIT", "41"))
# graduated split sizes (in 128-row tiles); first splits small so the
# output DMA starts early. Must sum to NT (=64).
_SPLITS = {
    1: [64],
    2: [32, 32],
    3: [16, 24, 24],
    4: [16, 16, 16, 16],
    40: [8, 16, 16, 24],
    41: [8, 16, 40],
    42: [8, 24, 32],
    43: [12, 20, 32],
    44: [8, 8, 16, 32],
    45: [8, 16, 16, 16, 8],
    46: [4, 12, 16, 16, 16],
    47: [8, 8, 16, 16, 16],
    48: [8, 12, 12, 16, 16],
    50: [4, 20, 20, 20],
    51: [4, 12, 24, 24],
    52: [4, 16, 20, 24],
    53: [8, 16, 20, 20],
    54: [4, 8, 16, 36],
    55: [4, 16, 44],
}
INTERLEAVE = int(os.environ.get("BNET_ILV", "1"))
REPS = int(os.environ.get("BNET_REPS", "1"))
SPLIT_SIZES = _SPLITS[NSPLIT]
MM_RED = os.environ.get("BNET_MM_DT", "fp16")   # fp32 | fp32r | fp16 | bf16
OUT_RED = os.environ.get("BNET_OUT_DT", "fp16")  # fp32 | fp16
TILES_PER_BATCH = int(os.environ.get("BNET_TPB", "4"))
SPLIT_COL = int(os.environ.get("BNET_SPLIT_COL", "304"))
# PSUM->SBUF copy engine split: DVE takes bank A (cols 0:SPLIT_COL), ACT
# takes the first B_ACT cols of bank B, Pool the rest.
B_ACT = int(os.environ.get("BNET_B_ACT", "469"))
RELU1 = os.environ.get("BNET_RELU1", "act")    # act | dve
RELU2 = os.environ.get("BNET_RELU2", "dve")    # act | dve
WBUFS = int(os.environ.get("BNET_WBUFS", "2"))
# PSUM pool buffering: 8 banks total. psH(1 bank/buf) + psS(1) +
# psA(1) x2 + psB(2 banks/buf)
PSH = int(os.environ.get("BNET_PSH", "2"))
PSS = int(os.environ.get("BNET_PSS", "2"))
PSB = int(os.environ.get("BNET_PSB", "1"))
ODMA = os.environ.get("BNET_ODMA", "sp")      # sp | alt (stores on both HWDGE rings)
CB = N_COLS - SPLIT_COL

# f32 constant-blob columns: [pack(20) | zeros(512)]
BLOB_PACK, BLOB_ZERO = 0, 20
BLOB_W = 20 + 512
# matmul-dtype blob columns:
# [W1(128) | W2x(64) | RHS(773) | RXS(4) | EYE(128) | XB(32)]
BLOBR_W1, BLOBR_W2X, BLOBR_RHS = 0, 128, 192
BLOBR_RXS, BLOBR_EYE, BLOBR_XB = 965, 969, 1097
BLOBR_W = 1097 + 4

LAST_RESULTS = None  # populated by kernel() for test harness introspection


MMDT = {
    "fp32": dt.float32,
    "fp32r": dt.float32r,
    "fp16": dt.float16,
    "bf16": dt.bfloat16,
}[MM_RED]
OUTDT = dt.float16 if OUT_RED == "fp16" else dt.float32
# feature buffer dtype: 16-bit when the matmul path is 16-bit (1 cyc/row
# PE transposes), else f32
FBDT = MMDT if MM_RED in ("fp16", "bf16") else dt.float32


def _r(ap):
    """No-op: operands now carry the matmul dtype via their tiles."""
    return ap


class _one_act_table:
    """Scoped: expose only act-function sets covering ALL funcs this kernel
    uses (Sin/Tanh/Relu/Copy all live in e.g. silu_and_others), so
    insert_act_table_loads hoists a single LoadActFuncSet to program entry
    instead of ping-ponging between trig and tanh tables every pass."""

    NEEDED = {"sin", "tanh", "relu", "copy", "identity", "memset_zero"}

    def __enter__(self):
        import concourse.bacc as _bacc
        self._orig = _bacc.get_activation_tables

        def only_full(arch, _orig=self._orig):
            tabs = _orig(arch)
            AFT = mybir.ActivationFunctionType
            need = {AFT.from_pwp(n) for n in self.NEEDED}
            if not any(need <= v for v in tabs.values()):
                return tabs
            # keep dict size/order (act_func_set_id indexes the original
            # act_info.json list) but make non-covering sets unselectable
            return {k: (v if need <= v else set()) for k, v in tabs.items()}

        _bacc.get_activation_tables = only_full
        return self

    def __exit__(self, *exc):
        import concourse.bacc as _bacc
        _bacc.get_activation_tables = self._orig
        return False


def build_program():
    nc = bacc.Bacc(None)

    xj_d = nc.dram_tensor("xj", [P, NT * N_IN], dt.float32, kind="ExternalInput")
    xT_d = nc.dram_tensor("xT", [NXT, BS], MMDT, kind="ExternalInput")
    blob_d = nc.dram_tensor("blob", [P, BLOB_W], dt.float32, kind="ExternalInput")
    blobr_d = nc.dram_tensor("blobr", [P, BLOBR_W], MMDT, kind="ExternalInput")
    out_d = nc.dram_tensor("out", [BS, N_COLS], OUTDT, kind="ExternalOutput")

    CH = 512                     # MLP chunk free size
    NTS_MAX = max(SPLIT_SIZES)

    PI = float(np.pi)
    TWO_PI = float(2 * np.pi)
    INV2PI = float(1.0 / (2 * np.pi))

    with tile.TileContext(nc) as tc:
        with (
            tc.tile_pool(name="const", bufs=1) as cpool,
            tc.tile_pool(name="work", bufs=WBUFS) as wpool,
            tc.tile_pool(name="hidp", bufs=3) as hpool,
            tc.tile_pool(name="outp", bufs=2) as opool,
            tc.tile_pool(name="psH", bufs=PSH, space="PSUM") as psH_pool,
            tc.tile_pool(name="psA", bufs=2, space="PSUM") as psA_pool,
            tc.tile_pool(name="psS", bufs=PSS, space="PSUM") as psS_pool,
            tc.tile_pool(name="psB", bufs=PSB, space="PSUM") as psB_pool,
        ):
            # ---------------- constants (single blob DMA) ----------------
            blob_sb = cpool.tile([P, BLOB_W], dt.float32, name="blob_sb")
            nc.sync.dma_start(out=blob_sb[:], in_=blob_d[:])
            blobr_sb = cpool.tile([P, BLOBR_W], MMDT, name="blobr_sb")
            nc.sync.dma_start(out=blobr_sb[:], in_=blobr_d[:])
            W1_sb = blobr_sb[0:NXT, BLOBR_W1:BLOBR_W1 + H1]
            W2x_sb = blobr_sb[:, BLOBR_W2X:BLOBR_W2X + 2 * H2]
            RHS_sb = blobr_sb[0:K_F, BLOBR_RHS:BLOBR_RHS + N_COLS]
            pack_sb = blob_sb[:, BLOB_PACK:BLOB_PACK + 20]
            zeros_sb = blob_sb[:, BLOB_ZERO:BLOB_ZERO + 512]
            rxs_sb = blobr_sb[:, BLOBR_RXS:BLOBR_RXS + 4]
            eye_sb = blobr_sb[:, BLOBR_EYE:BLOBR_EYE + P]
            xb_sb = blobr_sb[0:1, BLOBR_XB:BLOBR_XB + 4]
            xT_sb = cpool.tile([NXT, BS], MMDT, name="xT_sb")
            nc.sync.dma_start(out=xT_sb[:], in_=xT_d[:])

            b2x_ap = pack_sb[:, 17:18]

            j0 = 0
            state = {}

            def stage_A_gen(s):
                NTs = SPLIT_SIZES[s]
                n_ch = max(1, NTs * P // CH)
                j0 = sum(SPLIT_SIZES[:s])
                # per-split buffers; same names across splits -> shared
                # slots, multi-buffered by the pool
                xj_sb = wpool.tile(
                    [P, NTS_MAX * N_IN], dt.float32, name="xj_sb"
                )[:, 0:NTs * N_IN]
                x0_sb = wpool.tile(
                    [P, NTS_MAX * N_IN], dt.float32, name="x0_sb"
                )[:, 0:NTs * N_IN]
                # x21|x22 for two MLP chunks packed on the partition axis
                # (even chunk rows 0:64, odd chunk rows 64:128) so relu2
                # drains [128, CH] per instruction instead of [64, CH]
                x2s = wpool.tile(
                    [2 * 2 * H2, (NTS_MAX // 2) * P], MMDT, name="x2s"
                )[:, 0:(NTs // 2) * P]
                sigb = wpool.tile(
                    [P, 2 * NTS_MAX], dt.float32, name="sigb"
                )[:, 0:2 * NTs]
                Fbuf = wpool.tile(
                    [P, NTS_MAX * F_SLOTS], FBDT, name="Fbuf"
                )[:, 0:NTs * F_SLOTS]
                FT = wpool.tile(
                    [F_SLOTS, NTS_MAX * P], MMDT, name="FT"
                )[:, 0:NTs * P]

                nc.gpsimd.dma_start(
                    out=xj_sb[:], in_=xj_d[:, j0 * N_IN:(j0 + NTs) * N_IN]
                )

                FbV = Fbuf.rearrange("p (j f) -> p j f", f=F_SLOTS)

                def Fk(k):
                    return FbV[:, :, k]

                def T(nm):
                    return wpool.tile(
                        [P, NTS_MAX], dt.float32, name=nm, tag=nm
                    )[:, 0:NTs]

                V = nc.vector
                G = nc.gpsimd

                # ---------- sigma-independent row features ----------
                # (emitted before the MLP: DVE/GPSIMD fill the window in
                # which ACT/PE run the MLP)
                xjr = xj_sb.rearrange("p (j c) -> p j c", c=N_IN)
                x0r = x0_sb.rearrange("p (j c) -> p j c", c=N_IN)
                for cc_ in range(N_IN):
                    V.tensor_scalar(
                        x0r[:, :, cc_], xjr[:, :, cc_],
                        pack_sb[:, 1 + cc_:2 + cc_], pack_sb[:, 9 + cc_:10 + cc_],
                        ALU.mult, ALU.add,
                    )

                px = x0r[:, :, 0]
                py = x0r[:, :, 1]
                th = x0r[:, :, 2]
                v = x0r[:, :, 3]
                qx = x0r[:, :, 4]
                qy = x0r[:, :, 5]
                qth = x0r[:, :, 6]
                qv = x0r[:, :, 7]

                # Range-reduce angles into [-pi, pi] (no mod op on DVE):
                # n ~ round(t/2pi) via f32->i32 convert (works for either
                # truncation or round-to-nearest), then fix the residual
                # with two compare-and-subtract steps.
                rr_t = T("rr_t")
                rr_n = wpool.tile(
                    [P, NTS_MAX], dt.int32, name="rr_n", tag="rr_n"
                )[:, 0:NTs]
                rr_nf = T("rr_nf")
                rr_c = T("rr_c")
                rr_b = T("rr_b")

                def reduce_angle(dst, theta_ap, quarter):
                    # dst <- theta reduced to [-pi, pi]
                    V.tensor_scalar(rr_t[:], theta_ap, INV2PI, quarter,
                                    ALU.mult, ALU.add)
                    V.tensor_copy(rr_n[:], rr_t[:])
                    V.tensor_copy(rr_nf[:], rr_n[:])
                    V.scalar_tensor_tensor(dst, rr_nf[:], -TWO_PI, theta_ap,
                                           ALU.mult, ALU.add)
                    V.tensor_scalar(rr_c[:], dst, PI, None, ALU.is_gt)
                    V.scalar_tensor_tensor(dst, rr_c[:], -TWO_PI, dst,
                                           ALU.mult, ALU.add)
                    V.tensor_scalar(rr_c[:], dst, -PI, None, ALU.is_lt)
                    V.scalar_tensor_tensor(dst, rr_c[:], TWO_PI, dst,
                                           ALU.mult, ALU.add)

                def sincos_act(dsts, dstc, xred, sq):
                    # dsts <- sin(xred), dstc <- cos(xred); xred in [-pi, pi]
                    # ACT Sin table is accurate to |x| <~ 4.18; cos goes
                    # through the half-angle identity to stay in domain.
                    # The square runs on GPSIMD (SBUF-only, idle engine).
                    nc.scalar.activation(dsts, xred, AF.Sin)
                    nc.scalar.activation(sq[:], xred, AF.Sin, scale=0.5)
                    G.tensor_tensor(sq[:], sq[:], sq[:], ALU.mult)
                    V.tensor_scalar(dstc, sq[:], -2.0, 1.0, ALU.mult, ALU.add)

                sn = Fk(5)
                cs = Fk(4)
                so = T("so")
                co = T("co")
                thr = T("thr")
                thro = T("thro")
                sq0 = T("sq0")
                sq1 = T("sq1")
                reduce_angle(thr[:], th, 0.0)
                reduce_angle(thro[:], qth, 0.0)
                sincos_act(sn, cs, thr[:], sq0)
                sincos_act(so[:], co[:], thro[:], sq1)

                a0 = Fk(1)
                a1 = Fk(2)
                V.scalar_tensor_tensor(a0, v, 2.0, cs, ALU.mult, ALU.mult)
                V.scalar_tensor_tensor(a1, v, 2.0, sn, ALU.mult, ALU.mult)
                t0 = T("t0")
                t1 = T("t1")
                t2 = T("t2")
                t3 = T("t3")
                # pure tensor-tensor combos run on GPSIMD (SBUF-only ops
                # are legal there and the engine is otherwise idle)
                # f0 = a1*px - a0*py
                G.tensor_tensor(t0[:], a1, px, ALU.mult)
                G.tensor_tensor(t1[:], a0, py, ALU.mult)
                G.tensor_tensor(Fk(0), t0[:], t1[:], ALU.subtract)
                # f3 = c*px + s*py   (RHS carries the -2 factor)
                G.tensor_tensor(t2[:], cs, px, ALU.mult)
                G.tensor_tensor(t3[:], sn, py, ALU.mult)
                G.tensor_tensor(Fk(3), t2[:], t3[:], ALU.add)
                # vv = v*v ; u = a0*px + a1*py ; w = px^2 + py^2 (persist)
                vv = T("vv")
                G.tensor_tensor(vv[:], v, v, ALU.mult)
                u_ = T("u_")
                G.tensor_tensor(t0[:], a0, px, ALU.mult)
                G.tensor_tensor(t1[:], a1, py, ALU.mult)
                G.tensor_tensor(u_[:], t0[:], t1[:], ALU.add)
                w_ = T("w_")
                G.tensor_tensor(t2[:], px, px, ALU.mult)
                G.tensor_tensor(t3[:], py, py, ALU.mult)
                G.tensor_tensor(w_[:], t2[:], t3[:], ALU.add)

                # opponent statics on GPSIMD (tensor_tensor only; the
                # Pool engine has no scalar-operand ALU op). Constants are
                # pre-scaled on DVE (qv2, v4, vv2) or folded into RHS
                # (the -Ro^2 term of ob goes to RHS[9, 772]).
                qv2 = T("qv2")
                V.tensor_scalar(qv2[:], qv, 2.0, None, ALU.mult)
                v4 = T("v4")
                V.tensor_scalar(v4[:], v, -4.0, None, ALU.mult)
                vv2 = T("vv2")
                V.tensor_scalar(vv2[:], vv[:], 2.0, None, ALU.mult)
                g0 = T("g0")
                g1 = T("g1")
                dxo = T("dxo")
                dyo = T("dyo")
                G.tensor_tensor(dxo[:], px, qx, ALU.subtract)
                G.tensor_tensor(dyo[:], py, qy, ALU.subtract)
                G.tensor_tensor(g0[:], a1, dxo[:], ALU.mult)
                G.tensor_tensor(g1[:], a0, dyo[:], ALU.mult)
                G.tensor_tensor(Fk(10), g0[:], g1[:], ALU.subtract)
                G.tensor_tensor(g0[:], cs, dxo[:], ALU.mult)
                G.tensor_tensor(g1[:], sn, dyo[:], ALU.mult)
                G.tensor_tensor(Fk(11), g0[:], g1[:], ALU.add)
                ob_ = T("ob_")
                G.tensor_tensor(g0[:], dxo[:], dxo[:], ALU.mult)
                G.tensor_tensor(g1[:], dyo[:], dyo[:], ALU.mult)
                G.tensor_tensor(ob_[:], g0[:], g1[:], ALU.add)
                obd_ = T("obd_")
                G.tensor_tensor(g0[:], qv2[:], co[:], ALU.mult)
                G.tensor_tensor(g0[:], a0, g0[:], ALU.subtract)
                G.tensor_tensor(g0[:], dxo[:], g0[:], ALU.mult)
                G.tensor_tensor(g1[:], qv2[:], so[:], ALU.mult)
                G.tensor_tensor(g1[:], a1, g1[:], ALU.subtract)
                G.tensor_tensor(g1[:], dyo[:], g1[:], ALU.mult)
                G.tensor_tensor(obd_[:], g0[:], g1[:], ALU.add)
                olf_ = T("olf_")
                G.tensor_tensor(g0[:], cs, co[:], ALU.mult)
                G.tensor_tensor(g1[:], sn, so[:], ALU.mult)
                G.tensor_tensor(g0[:], g0[:], g1[:], ALU.subtract)
                G.tensor_tensor(g1[:], v4[:], qv, ALU.mult)
                G.tensor_tensor(g0[:], g1[:], g0[:], ALU.mult)
                G.tensor_tensor(g1[:], qv2[:], qv, ALU.mult)
                G.tensor_tensor(g1[:], g1[:], g0[:], ALU.add)
                G.tensor_tensor(olf_[:], vv2[:], g1[:], ALU.add)

                yield  # --- part boundary: features emitted ---

                # ---------------- phase A: MLP ----------------
                # b1 is folded into W1 via the xT ones row (no relu bias)
                for c in range(n_ch):
                    col0 = c * CH
                    xcol0 = j0 * P + col0
                    ps_h = psH_pool.tile(
                        [P, CH], dt.float32, name="ps_h", tag="ps_h"
                    )
                    nc.tensor.matmul(
                        ps_h[:], _r(W1_sb[:]), _r(xT_sb[:, xcol0:xcol0 + CH])
                    )
                    hid = hpool.tile([P, CH], MMDT, name="hid")
                    if RELU1 == "dve":
                        nc.vector.tensor_scalar(
                            hid[:], ps_h[:], 0.0, None, ALU.max
                        )
                    else:
                        nc.scalar.activation(hid[:], ps_h[:], AF.Relu)
                    half = c % 2
                    if half == 0:
                        ps2x = psS_pool.tile(
                            [P, CH], dt.float32, name="ps2x", tag="ps_sm"
                        )
                    nc.tensor.matmul(
                        ps2x[64 * half:64 * half + 2 * H2, :],
                        _r(W2x_sb[:]), _r(hid[:])
                    )
                    if half == 1:
                        # one relu for 2 chunks of stacked x21|x22 with
                        # per-partition biases (b2x repeated on rows 64:128)
                        xcols = (c // 2) * CH
                        if RELU2 == "dve":
                            nc.vector.tensor_scalar(
                                x2s[:, xcols:xcols + CH], ps2x[:],
                                b2x_ap, 0.0, ALU.add, ALU.max
                            )
                        else:
                            nc.scalar.activation(
                                x2s[:, xcols:xcols + CH], ps2x[:], AF.Relu,
                                bias=b2x_ap,
                            )

                yield  # --- part boundary: MLP emitted ---

                # -------- phase A2: x31 + sigma, 8-tile groups --------
                # biases (b31|b32 per tile) enter via a rank-1 accumulate
                # matmul against the xT ones row — no ones row in x2s.
                sgv = sigb.rearrange("p (j t) -> p j t", t=2)
                for g in range(NTs // 8):
                    ps_xs = psS_pool.tile(
                        [P, 32], dt.float32, name="ps_xs", tag="ps_sm"
                    )
                    for jp in range(4):
                        jj = 8 * g + 2 * jp
                        for k in (0, 1):
                            reg = ps_xs[:, jp * 8 + 4 * k:jp * 8 + 4 * k + 4]
                            nc.tensor.matmul(
                                reg, _r(xT_sb[0:1, 0:P]), _r(xb_sb[:]),
                                start=True, stop=False,
                            )
                            # packed x2s: tile t lives on partition half
                            # (t//4)%2, cols ((t//4)//2)*CH + (t%4)*P
                            t = jj + k
                            hf = (t // 4) % 2
                            xc = ((t // 4) // 2) * CH + (t % 4) * P
                            nc.tensor.matmul(
                                reg,
                                _r(x2s[64 * hf:64 * hf + 2 * H2, xc:xc + P]),
                                _r(rxs_sb[64 * hf:64 * hf + 2 * H2, :]),
                                start=False, stop=True,
                            )
                    pg = ps_xs.rearrange("p (j k) -> p j k", k=4)
                    # sigma = 4*sigmoid(z) enters downstream only through
                    # sbar/q; store t = tanh(z/2) (same act table as Sin)
                    nc.scalar.activation(
                        sgv[:, 8 * g:8 * g + 8, :], pg[:, :, 2:4], AF.Tanh,
                        scale=0.5,
                    )
                    nc.scalar.copy(
                        FbV[:, 8 * g:8 * g + 8, 13:15], pg[:, :, 0:2]
                    )

                # ---------- sigma-dependent features (DVE) ----------
                # p_i = 4*sigmoid(z_i) = 2 + 2*t_i with t_i = tanh(z_i/2):
                # sbar = p0+p1 = 4 + 2*(t0+t1); q = p0*p1 = 4*(1+t0)(1+t1)
                sgr = sigb.rearrange("p (j t) -> p j t", t=2)
                sg0 = sgr[:, :, 0]
                sg1 = sgr[:, :, 1]
                q_ = Fk(9)
                sb_ = T("sb_")
                tp_ = T("tp_")
                G.tensor_tensor(sb_[:], sg0, sg1, ALU.add)      # S = t0+t1
                G.tensor_tensor(tp_[:], sg0, sg1, ALU.mult)     # P = t0*t1
                G.tensor_tensor(tp_[:], tp_[:], sb_[:], ALU.add)
                V.tensor_scalar(q_, tp_[:], 4.0, 4.0, ALU.mult, ALU.add)
                V.tensor_scalar(sb_[:], sb_[:], 2.0, 4.0, ALU.mult, ALU.add)
                # f6 = 2*vv + sbar*u + q*w
                G.tensor_tensor(g0[:], sb_[:], u_[:], ALU.mult)
                G.tensor_tensor(g1[:], q_, w_[:], ALU.mult)
                G.tensor_tensor(g0[:], g0[:], g1[:], ALU.add)
                V.scalar_tensor_tensor(Fk(6), vv[:], 2.0, g0[:], ALU.mult, ALU.add)
                # f7 = sbar*a0 + 2*q*px ; f8 = sbar*a1 + 2*q*py
                G.tensor_tensor(t0[:], sb_[:], a0, ALU.mult)
                V.scalar_tensor_tensor(g1[:], q_, 2.0, px, ALU.mult, ALU.mult)
                G.tensor_tensor(Fk(7), t0[:], g1[:], ALU.add)
                G.tensor_tensor(t1[:], sb_[:], a1, ALU.mult)
                V.scalar_tensor_tensor(t2[:], q_, 2.0, py, ALU.mult, ALU.mult)
                G.tensor_tensor(Fk(8), t1[:], t2[:], ALU.add)
                # f12 = olf + sbar*obd + q*ob
                G.tensor_tensor(t3[:], sb_[:], obd_[:], ALU.mult)
                G.tensor_tensor(g0[:], q_, ob_[:], ALU.mult)
                G.tensor_tensor(t3[:], t3[:], g0[:], ALU.add)
                G.tensor_tensor(Fk(12), t3[:], olf_[:], ALU.add)
                state[s] = (NTs, j0, FbV, FT, ob_)

            def stage_C(s, bbs=None):
                NTs, j0, FbV, FT, _ = state[s]

                # ---------------- phase C: output tiles ----------------
                if bbs is None:
                    bbs = range(NTs // TILES_PER_BATCH)
                for bb in bbs:
                    ob_t = opool.tile(
                        [P, TILES_PER_BATCH, N_COLS], OUTDT, name="ob_t"
                    )
                    for jp in range(TILES_PER_BATCH // 2):
                        ji = 2 * jp
                        jj = bb * TILES_PER_BATCH + ji
                        # transpose feature pairs into lhsT layout
                        ps_tr = psS_pool.tile(
                            [K_F, 2 * P], FBDT, name="ps_tr", tag="ps_sm"
                        )
                        nc.tensor.transpose(
                            ps_tr[:, 0:P], FbV[:, jj, 0:K_F], eye_sb[:]
                        )
                        nc.tensor.transpose(
                            ps_tr[:, P:2 * P], FbV[:, jj + 1, 0:K_F], eye_sb[:]
                        )
                        nc.vector.tensor_copy(
                            FT[0:K_F, jj * P:(jj + 2) * P], ps_tr[:]
                        )
                        # 512-aligned segments: each mmB must stay
                        # inside one PSUM bank
                        psB = psB_pool.tile(
                            [P, 1024], dt.float32, name="psB", tag="psB"
                        )
                        for k in (0, 1):
                            lhsF = FT[0:K_F, (jj + k) * P:(jj + k + 1) * P]
                            psA = psA_pool.tile(
                                [P, SPLIT_COL], dt.float32, name="psA",
                                tag="psA",
                            )
                            nc.tensor.matmul(
                                psA[:], _r(lhsF), _r(RHS_sb[:, 0:SPLIT_COL])
                            )
                            nc.tensor.matmul(
                                psB[:, k * 512:k * 512 + CB], _r(lhsF),
                                _r(RHS_sb[:, SPLIT_COL:N_COLS])
                            )
                            nc.vector.tensor_copy(
                                ob_t[:, ji + k, 0:SPLIT_COL], psA[:]
                            )
                        pbv = psB.rearrange("p (j c) -> p j c", c=512)
                        nc.scalar.copy(
                            ob_t[:, ji:ji + 2, SPLIT_COL:SPLIT_COL + B_ACT],
                            pbv[:, :, 0:B_ACT]
                        )
                        if B_ACT < CB:
                            nc.vector.tensor_copy(
                                ob_t[:, ji:ji + 2, SPLIT_COL + B_ACT:N_COLS],
                                pbv[:, :, B_ACT:CB]
                            )
                    r0 = (j0 + bb * TILES_PER_BATCH) * P
                    od = out_d[r0:r0 + TILES_PER_BATCH * P, :].rearrange(
                        "(jl p) c -> p jl c", p=P
                    )
                    eng = nc.sync
                    if ODMA == "alt" and (j0 // TILES_PER_BATCH + bb) % 2:
                        eng = nc.scalar
                    eng.dma_start(out=od, in_=ob_t[:])



            nsp = len(SPLIT_SIZES)

            for _rep in range(REPS):
                gens = {}
                parts_done = {}

                def stage_A(s, upto=3, gens=gens, parts_done=parts_done):
                    if s not in gens:
                        gens[s] = stage_A_gen(s)
                        parts_done[s] = 0
                    while parts_done[s] < upto:
                        next(gens[s], None)
                        parts_done[s] += 1

                if INTERLEAVE:
                    stage_A(0)
                    for s in range(nsp):
                        nb = SPLIT_SIZES[s] // TILES_PER_BATCH
                        nxt = s + 1 if s + 1 < nsp else None
                        if nxt:
                            stage_A(nxt, 1)
                        stage_C(s, [0])
                        if nxt:
                            stage_A(nxt, 2)
                        stage_C(s, [1] if nb > 1 else [])
                        if nxt:
                            stage_A(nxt, 3)
                        stage_C(s, range(2, nb))
                else:
                    for s in range(nsp):
                        stage_A(s)
                        stage_C(s)
    with _one_act_table():
        nc.finalize()
    return nc


def _host_prep(x, obstacles, input_mean, input_std,
               W1, b1, W21, b21, W22, b22, W31, b31, W32, b32):
    """Layout-only host prep; returns per-core input maps."""
    x = np.asarray(x, np.float32)
    obstacles = np.asarray(obstacles, np.float32)

    ox, oy, orr = obstacles[:, 0], obstacles[:, 1], obstacles[:, 2]
    R = 0.5 + orr + 0.1
    quad = ox * ox + oy * oy - R * R
    RHS = np.zeros((K_F, N_COLS), np.float32)
    c = np.arange(N_OBS)
    g1 = 2 + 2 * c
    g2 = 3 + 2 * c
    RHS[0, g1] = 1.0
    RHS[1, g1] = oy
    RHS[2, g1] = -ox
    RHS[3, g2] = -2.0
    RHS[4, g2] = 2.0 * ox
    RHS[5, g2] = 2.0 * oy
    RHS[10, 2 + 2 * N_OBS] = 1.0
    RHS[11, 3 + 2 * N_OBS] = -2.0
    hs = 2 + 2 * (N_OBS + 1) + c
    RHS[6, hs] = 1.0
    RHS[7, hs] = -ox
    RHS[8, hs] = -oy
    RHS[9, hs] = quad
    RHS[12, N_COLS - 1] = 1.0
    RHS[9, N_COLS - 1] = -1.21
    RHS[13, 0] = 1.0
    RHS[14, 1] = 1.0

    rxs = np.zeros((P, 4), np.float32)
    rxs[0:H2, 0:2] = np.asarray(W31, np.float32)
    rxs[H2:2 * H2, 2:4] = np.asarray(W32, np.float32)
    rxs[2 * H2:4 * H2] = rxs[0:2 * H2]   # packed odd-chunk mirror
    xb = np.concatenate([np.asarray(b31, np.float32),
                         np.asarray(b32, np.float32)])  # [4] per tile

    pack = np.zeros((P, 20), np.float32)
    pack[:, 0] = np.asarray(b1, np.float32)
    pack[:, 1:9] = np.asarray(input_std, np.float32)[None, :]
    pack[:, 9:17] = np.asarray(input_mean, np.float32)[None, :]
    b2x = np.concatenate([np.asarray(b21, np.float32),
                          np.asarray(b22, np.float32)])
    pack[0:2 * H2, 17] = b2x
    pack[2 * H2:4 * H2, 17] = b2x   # repeated for the packed odd chunk
    pack[:, 19] = -np.pi

    mmnp = mybir.dt.np(MMDT)
    blob = np.zeros((P, BLOB_W), np.float32)
    blob[:, BLOB_PACK:BLOB_PACK + 20] = pack
    # xT row 0 is the ones row (bias lane); W1's bias row leads to match
    blobr = np.zeros((P, BLOBR_W), np.float32)
    blobr[0, BLOBR_W1:BLOBR_W1 + H1] = np.asarray(b1, np.float32)
    blobr[1:NXT, BLOBR_W1:BLOBR_W1 + H1] = np.asarray(W1, np.float32)
    blobr[:, BLOBR_W2X:BLOBR_W2X + H2] = np.asarray(W21, np.float32)
    blobr[:, BLOBR_W2X + H2:BLOBR_W2X + 2 * H2] = np.asarray(W22, np.float32)
    blobr[0:K_F, BLOBR_RHS:BLOBR_RHS + N_COLS] = RHS
    blobr[:, BLOBR_RXS:BLOBR_RXS + 4] = rxs
    blobr[:, BLOBR_EYE:BLOBR_EYE + P] = np.eye(P, dtype=np.float32)
    blobr[0, BLOBR_XB:BLOBR_XB + 4] = xb
    blobr = np.ascontiguousarray(blobr.astype(mmnp))

    shared = dict(
        blob=blob,
        blobr=blobr,
    )

    per_core = []
    for i in range(N_CORES):
        xs = x[i * BS:(i + 1) * BS]
        xjm = np.ascontiguousarray(
            xs.reshape(NT, P, N_IN).transpose(1, 0, 2).reshape(P, NT * N_IN)
        )
        xTm = np.empty((NXT, BS), mmnp)
        xTm[0] = 1.0
        xTm[1:NXT] = xs.T.astype(mmnp)
        per_core.append(dict(shared, xj=xjm, xT=xTm))
    return per_core


def _host_post(arr):
    """Upcast the device's fp16 output back to f32 (layout unchanged)."""
    return np.ascontiguousarray(arr.astype(np.float32))


def kernel(x, obstacles, input_mean, input_std,
           W1, b1, W21, b21, W22, b22, W31, b31, W32, b32, sgn=None):
    global LAST_RESULTS
    nc = build_program()
    in_maps = _host_prep(x, obstacles, input_mean, input_std,
                         W1, b1, W21, b21, W22, b22, W31, b31, W32, b32)
    res = run_bass_kernel_spmd(
        nc, in_maps, core_ids=list(range(N_CORES)),
        trace=bool(int(os.environ.get("BNET_TRACE", "0"))),
    )
    LAST_RESULTS = res
    outs = [res.results[i]["out"] for i in range(N_CORES)]
    return _host_post(np.concatenate(outs, axis=0))

